# revision 24
# baseline (speedup 1.0000x reference)
"""AttentionPooling (segment softmax-weighted scatter) Trainium2 Bass kernel.

Strategy (8 NeuronCores, SPMD):
  - Shard by SEGMENT BLOCKS: core c owns segments [c*128, (c+1)*128) and all
    nodes whose (sorted) batch id falls in that range. No cross-core reduction
    is needed: each segment lives entirely on one core. Host pads each core's
    node count to a common T tiles of 128 so the compiled program is identical
    across cores.
  - Scores are computed without max-subtraction: p = exp(s + b2) directly.
    Scores are bounded (|tanh|<=1, |W2| small) so raw exp is safe in fp32,
    and the reference's +1e-8 epsilon is negligible relative to seg sums.
  - Phase A: s = tanh(x @ W1 + b1) @ W2 from a host-fed TRANSPOSED fp8 copy
    of 16*x. The 256-deep contraction runs as ONE DoubleRow fp8 matmul
    (hid-half pair on the DR axis) plus a second DR matmul against the fp8
    residual of 16*W1, restoring full W1 precision. tanh rescales by 1/256.
  - W2 scores for 4 consecutive chunks are steered to partitions 0..3 of one
    [4, F] PSUM tile (lhsT = w2 embedded in column j of a [H2, 4] zero block),
    one accumulation group, then a single DVE copy moves all 4 rows to SBUF.
  - Scores transpose to per-node columns directly from that SBUF tile (PE
    transposes, no DRAM round-trip); exp(s + b2) -> p_cols. scores also DMA
    to DRAM only for the host-side segment-sum export.
  - Phase B: one-hot weighted scatter. S[i, m] = (m == c_i) * p_i built by one
    dual-op tensor_scalar; out[seg, :] += S^T @ x accumulates in PSUM over all
    T tiles (natural-layout bf16 x, fed in X4G-tile batched DMAs with paced
    prefetch so the DMA engines never starve).
  - Pipelining: uniform SB-chunk blocks; block b's phase-A chunks interleave
    with block b-1's phase-B tiles (FB tiles per chunk), so scatter lags
    scores by only SB chunks at both ends of the program.
  - Host divides wx by the sum column and concatenates core outputs.
"""

from functools import lru_cache

import ml_dtypes
import numpy as np

import concourse.mybir as mybir
import concourse.tile as tile
from concourse import bacc
from concourse.masks import make_identity

P = 128          # partitions / tile rows
HID = 256        # hidden dim
H2 = 128         # MLP inner dim
NSEG = 1024      # segments (batch size)
NCORES = 8
F = 512          # phase-A chunk (nodes per score chunk)
FB = F // P      # tiles per chunk

BF16 = mybir.dt.bfloat16
FP8 = mybir.dt.float8e4
F32 = mybir.dt.float32
NPBF16 = ml_dtypes.bfloat16
NPFP8 = ml_dtypes.float8_e4m3
XSCALE = 16.0    # x and W1 are fed as fp8(16*v); tanh rescales by 1/256

SB = 16          # chunks per block / sub-bridge
XTP = 8          # chunks per xT DMA
X4G = 8          # node-tiles per natural-x DMA
X4_BUFS = 16     # natural-x tile buffers ([P, X4G*HID] bf16 each)
KB = 4           # x4-groups of phase-B emitted per phase-A block


def tile_order(chunks: int) -> np.ndarray:
    """Device iteration order: tile index t for each phase-B step j."""
    sb = min(SB, chunks)
    j = np.arange(chunks * FB)
    blk, s = j // (FB * sb), j % (FB * sb)
    fb, nl = s // sb, s % sb
    return FB * (blk * sb + nl) + fb


def build_kernel(chunks: int):
    assert chunks % SB == 0 and chunks % XTP == 0 and chunks <= P
    NB = chunks // SB
    T = chunks * FB
    n_pad = T * P

    nc = bacc.Bacc("TRN2")
    # natural x, X4G node-tiles per row: x[j, p, i*HID:(i+1)*HID] is lane p of
    # node-tile t_order[X4G*j + i]
    x_in = nc.dram_tensor("x", [T // X4G, P, X4G * HID], BF16,
                          kind="ExternalInput")
    # xT fp8: [128(k), chunks, 2(hid half i), F] = fp8(16*x[g*F+f, i*128+k])
    xT_in = nc.dram_tensor("xT", [P, chunks, 2, F], FP8, kind="ExternalInput")
    c_in = nc.dram_tensor("c", [P, T], F32, kind="ExternalInput")
    # w1: [2(hi/residual), 128(k), 2(hid half i), H2] fp8 of 16*W1 (+ residual)
    w1_in = nc.dram_tensor("w1", [2, P, 2, H2], FP8, kind="ExternalInput")
    # w2 on the diagonal of SB [H2, SB] blocks: lhsT w2_in[:, j] steers chunk
    # j%SB's scores to PSUM partition j of a shared [SB, F] tile.
    w2_in = nc.dram_tensor("w2", [H2, SB, SB], BF16, kind="ExternalInput")
    b1_in = nc.dram_tensor("b1", [H2, 1], F32, kind="ExternalInput")
    b2_in = nc.dram_tensor("b2", [P, 1], F32, kind="ExternalInput")
    iota_in = nc.dram_tensor("iota", [P, P], BF16, kind="ExternalInput")
    out_t = nc.dram_tensor("out", [P, HID], F32, kind="ExternalOutput")
    scores_d = nc.dram_tensor("scores", [chunks, F], F32, kind="ExternalOutput")

    with tile.TileContext(nc) as tc:
        with (
            tc.tile_pool(name="const", bufs=1) as cpool,
            tc.tile_pool(name="xT", bufs=6) as xT_pool,
            tc.tile_pool(name="th", bufs=4) as th_pool,
            tc.tile_pool(name="b16", bufs=3) as b_pool,
            tc.tile_pool(name="x4", bufs=X4_BUFS) as x4_pool,
            tc.tile_pool(name="S", bufs=8) as s_pool,
            tc.tile_pool(name="ph", bufs=2, space="PSUM") as ph_pool,
            tc.tile_pool(name="ps", bufs=2, space="PSUM") as ps_pool,
            tc.tile_pool(name="pT", bufs=1, space="PSUM") as pT_pool,
            tc.tile_pool(name="po", bufs=1, space="PSUM") as po_pool,
        ):
            # ---- constants ----
            w1h = cpool.tile([P, 2, H2], FP8, tag="w1h")
            w1r = cpool.tile([P, 2, H2], FP8, tag="w1r")
            w2q = cpool.tile([H2, SB, SB], BF16, tag="w2q")
            b1t = cpool.tile([H2, 1], F32, tag="b1t")
            b2t = cpool.tile([P, 1], F32, tag="b2t")
            iota_t = cpool.tile([P, P], BF16, tag="iota")
            ident = cpool.tile([P, P], F32, tag="ident")
            c_cols = cpool.tile([P, T], F32, tag="ccols")
            p_cols = cpool.tile([P, T], F32, tag="pcols")
            out_sb = cpool.tile([P, HID], F32, tag="osb")

            nc.gpsimd.dma_start(out=w1h[:], in_=w1_in[0])
            nc.gpsimd.dma_start(out=w1r[:], in_=w1_in[1])
            nc.gpsimd.dma_start(out=w2q[:], in_=w2_in[:])
            nc.gpsimd.dma_start(out=b1t[:], in_=b1_in[:])
            nc.gpsimd.dma_start(out=b2t[:], in_=b2_in[:])
            nc.gpsimd.dma_start(out=iota_t[:], in_=iota_in[:])
            nc.gpsimd.dma_start(out=c_cols[:], in_=c_in[:])
            make_identity(nc, ident[:])

            po = po_pool.tile([P, HID], F32)

            st = {"x4_load": 0, "x4": {}, "xT_load": 0, "xT": {},
                  "ps": None, "b16": None, "done": 0}

            def ensure_x4(upto: int, max_issue: int = 2):
                lim = min(upto, T // X4G, st["x4_load"] + max_issue)
                while st["x4_load"] < lim:
                    t = x4_pool.tile([P, X4G * HID], BF16, tag="x4", name="x4")
                    nc.sync.dma_start(out=t[:], in_=x_in[st["x4_load"]])
                    st["x4"][st["x4_load"]] = t
                    st["x4_load"] += 1

            def ensure_xT(upto: int, max_issue: int = 2):
                lim = min(upto, chunks // XTP, st["xT_load"] + max_issue)
                while st["xT_load"] < lim:
                    t = xT_pool.tile([P, XTP, 2, F], FP8, tag="xT", name="xT")
                    nc.sync.dma_start(
                        out=t[:],
                        in_=xT_in[:, st["xT_load"] * XTP:(st["xT_load"] + 1) * XTP])
                    st["xT"][st["xT_load"]] = t
                    st["xT_load"] += 1

            def phase_a_chunk(g):
                if g % XTP == 0:
                    ensure_xT(g // XTP + 2)
                    if g // XTP - 1 in st["xT"]:
                        del st["xT"][g // XTP - 1]
                xt = st["xT"][g // XTP]
                half = g % 2
                if half == 0:
                    st["ph2"] = ph_pool.tile([P, 2 * F], F32, tag="ph2",
                                             name="ph2")
                ph2 = st["ph2"]
                dst = ph2[:, half * F:(half + 1) * F]
                nc.tensor.matmul(out=dst, lhsT=w1h[:], rhs=xt[:, g % XTP],
                                 start=True, stop=False,
                                 perf_mode=mybir.MatmulPerfMode.DoubleRow)
                nc.tensor.matmul(out=dst, lhsT=w1r[:], rhs=xt[:, g % XTP],
                                 start=False, stop=True,
                                 perf_mode=mybir.MatmulPerfMode.DoubleRow)
                del xt
                if half == 0:
                    st["th2"] = th_pool.tile([P, 2 * F], BF16, tag="th2",
                                             name="th2")
                    return
                th2 = st["th2"]
                nc.scalar.activation(out=th2[:], in_=ph2[:],
                                     func=mybir.ActivationFunctionType.Tanh,
                                     bias=b1t[:], scale=1.0 / (XSCALE * XSCALE))
                for h in (0, 1):
                    j = (g - 1 + h) % SB
                    if j == 0:
                        st["ps"] = ps_pool.tile([SB, F], F32, tag="ps",
                                                name="ps")
                    ps = st["ps"]
                    nc.tensor.matmul(out=ps[:], lhsT=w2q[:, j],
                                     rhs=th2[:, h * F:(h + 1) * F],
                                     start=(j == 0), stop=(j == SB - 1),
                                     skip_group_check=True)
                if (g + 1) % SB == 0:
                    st["b16"] = b_pool.tile([SB, F], F32, tag="b16",
                                            name="b16")
                    nc.vector.tensor_copy(out=st["b16"][:], in_=ps[:])

            def sub_bridge(b):
                """Block b's scores -> p_cols (and DRAM export for the host)."""
                g_lo = b * SB
                b16 = st["b16"]
                nc.gpsimd.dma_start(out=scores_d[g_lo:g_lo + SB, :], in_=b16[:])
                pT = pT_pool.tile([P, FB * SB], F32)
                for fb in range(FB):
                    nc.tensor.transpose(
                        out=pT[:, fb * SB:(fb + 1) * SB],
                        in_=b16[:, fb * P:(fb + 1) * P],
                        identity=ident[:SB, :SB])
                nc.scalar.activation(
                    out=p_cols[:, g_lo * FB:(g_lo + SB) * FB], in_=pT[:],
                    func=mybir.ActivationFunctionType.Exp, bias=b2t[:],
                    scale=1.0)

            def phase_b_tiles(js):
                for j in js:
                    grp = j // X4G
                    x4 = st["x4"][grp]
                    if j % X4G == X4G - 1:
                        del st["x4"][grp]
                    i8 = j % X4G
                    S = s_pool.tile([P, P], BF16, tag="S")
                    nc.vector.tensor_scalar(
                        out=S[:], in0=iota_t[:],
                        scalar1=c_cols[:, j:j + 1], scalar2=p_cols[:, j:j + 1],
                        op0=mybir.AluOpType.is_equal, op1=mybir.AluOpType.mult)
                    nc.tensor.matmul(out=po[:], lhsT=S[:],
                                     rhs=x4[:, i8 * HID:(i8 + 1) * HID],
                                     start=(j == 0), stop=(j == T - 1),
                                     skip_group_check=True)

            def emit_b_group():
                d = st["done"]
                if d >= T:
                    return
                ensure_x4(d // X4G + 8)
                phase_b_tiles(range(d, d + X4G))
                st["done"] = d + X4G

            # Phase A front-loaded (Act/DMA paced) with just enough phase B
            # interleaved to keep the DMA engines fed; the rest of phase B
            # streams afterwards at x4-DMA pace.
            for b in range(NB):
                for gi in range(SB):
                    phase_a_chunk(b * SB + gi)
                    if gi == 1 and b > 0:
                        sub_bridge(b - 1)
                    if b > 0 and gi % 4 == 3:
                        emit_b_group()
            sub_bridge(NB - 1)
            while st["done"] < T:
                emit_b_group()

            nc.vector.tensor_copy(out=out_sb[:], in_=po[:])
            nc.gpsimd.dma_start(out=out_t[:], in_=out_sb[:])

    nc.finalize()
    return nc


@lru_cache(maxsize=4)
def _compiled(chunks: int):
    return build_kernel(chunks)


@lru_cache(maxsize=4)
def _runner(chunks: int):
    """Persistent jitted shard_map over the 8 cores (compiles once)."""
    import jax
    from concourse import bass2jax
    from jax.sharding import Mesh, PartitionSpec
    from jax.experimental.shard_map import shard_map

    nc = _compiled(chunks)
    bass2jax.install_neuronx_cc_hook()
    partition_name = nc.partition_id_tensor.name if nc.partition_id_tensor else None
    in_names, out_names, out_avals, zero_outs = [], [], [], []
    for alloc in nc.m.functions[0].allocations:
        if not isinstance(alloc, mybir.MemoryLocationSet):
            continue
        name = alloc.memorylocations[0].name
        if alloc.kind == "ExternalInput":
            if name != partition_name:
                in_names.append(name)
        elif alloc.kind == "ExternalOutput":
            out_names.append(name)
            shape = tuple(alloc.tensor_shape)
            dtype = mybir.dt.np(alloc.dtype)
            out_avals.append(jax.core.ShapedArray(shape, dtype))
            zero_outs.append(np.zeros(shape, dtype))
    n_params = len(in_names)
    all_in_names = list(in_names) + list(out_names)
    if partition_name is not None:
        all_in_names.append(partition_name)

    def _body(*args):
        operands = list(args)
        if partition_name is not None:
            operands.append(bass2jax.partition_id_tensor())
        outs = bass2jax._bass_exec_p.bind(
            *operands,
            out_avals=tuple(out_avals),
            in_names=tuple(all_in_names),
            out_names=tuple(out_names),
            lowering_input_output_aliases=(),
            sim_require_finite=True,
            sim_require_nnan=True,
            nc=nc,
        )
        return tuple(outs)

    devices = jax.devices()[:NCORES]
    assert len(devices) >= NCORES
    mesh = Mesh(np.asarray(devices), ("core",))
    in_specs = (PartitionSpec("core"),) * (n_params + len(out_names))
    out_specs = (PartitionSpec("core"),) * len(out_names)
    sharded = jax.jit(
        shard_map(_body, mesh=mesh, in_specs=in_specs, out_specs=out_specs,
                  check_rep=False),
        keep_unused=True,
    )
    concat_zeros = [
        np.zeros((NCORES * z.shape[0], *z.shape[1:]), z.dtype) for z in zero_outs
    ]

    def run(in_maps):
        concat_in = [
            np.concatenate([np.asarray(in_maps[c][n]) for c in range(NCORES)],
                           axis=0)
            for n in in_names
        ]
        out = sharded(*concat_in, *concat_zeros)
        return {
            name: np.asarray(out[i]).reshape(NCORES, *out_avals[i].shape)
            for i, name in enumerate(out_names)
        }

    return run


def _prep_inputs(x, batch, W1, b1, W2, b2):
    """Shard by segment blocks; build padded per-core arrays."""
    x = np.asarray(x, dtype=np.float32)
    batch = np.asarray(batch).astype(np.int64)
    bounds = np.searchsorted(batch, np.arange(0, NSEG + 1, P))
    counts = np.diff(bounds)
    maxn = int(counts.max())
    chunks = -(-maxn // F)
    chunks = -(-chunks // SB) * SB  # block alignment
    assert chunks <= P, f"core node count {maxn} exceeds capacity"
    T = chunks * FB
    n_pad = T * P
    t_order = tile_order(chunks)

    x_dev = np.zeros((NCORES, T // X4G, P, X4G * HID), dtype=NPBF16)
    xT_dev = np.zeros((NCORES, P, chunks, 2, F), dtype=NPFP8)
    c_dev = np.empty((NCORES, P, T), dtype=np.float32)
    for core in range(NCORES):
        s, e = int(bounds[core]), int(bounds[core + 1])
        n = e - s
        xs = x[s:e].astype(NPBF16)
        x_pad = np.zeros((n_pad, HID), dtype=NPBF16)
        x_pad[:n] = xs
        x_dev[core] = (x_pad.reshape(T, P, HID)[t_order]
                       .reshape(T // X4G, X4G, P, HID)
                       .transpose(0, 2, 1, 3)
                       .reshape(T // X4G, P, X4G * HID))
        x8_pad = np.zeros((n_pad, HID), dtype=NPFP8)
        x8_pad[:n] = (XSCALE * x[s:e]).astype(NPFP8)
        # [node(g,f), hid(i,k)] -> [k, g, i, f]
        xT_dev[core] = (x8_pad.reshape(chunks, F, 2, P)
                        .transpose(3, 0, 2, 1))
        c_all = np.full(n_pad, -1000.0, dtype=np.float32)
        c_all[:n] = (batch[s:e] - core * P).astype(np.float32)
        c_dev[core] = c_all.reshape(T, P)[t_order].T

    w1s = XSCALE * np.asarray(W1, dtype=np.float32)
    w1hi = w1s.astype(NPFP8)
    w1re = (w1s - w1hi.astype(np.float32)).astype(NPFP8)
    # [hid(i,k), m] -> [hi/res, k, i, m]
    w1 = (np.stack([w1hi, w1re])
          .reshape(2, 2, P, H2).transpose(0, 2, 1, 3))
    w2b = np.asarray(W2, dtype=np.float32).astype(NPBF16).reshape(H2)
    w2 = np.zeros((H2, SB, SB), dtype=NPBF16)
    for j in range(SB):
        w2[:, j, j] = w2b
    b1c = np.asarray(b1, dtype=np.float32).reshape(H2, 1)
    b2c = np.full((P, 1), np.float32(np.asarray(b2).reshape(-1)[0]))
    iota = np.broadcast_to(np.arange(P, dtype=np.float32), (P, P)).astype(NPBF16)

    in_maps = []
    for core in range(NCORES):
        in_maps.append({
            "x": x_dev[core], "xT": xT_dev[core], "c": c_dev[core],
            "w1": w1, "w2": w2, "b1": b1c, "b2": b2c, "iota": iota,
        })
    return chunks, in_maps


def _host_ssum(scores, batch, b2):
    """Per-segment sum of p = exp(score + b2), from exported per-core scores.

    scores[core] is [chunks, F] over that core's padded node stream; entry
    (g, f) is node g*F + f of the core's stream. Padded nodes are excluded by
    counting only the first n_c real nodes."""
    batch = np.asarray(batch).astype(np.int64)
    bounds = np.searchsorted(batch, np.arange(0, NSEG + 1, P))
    b2v = np.float32(np.asarray(b2, dtype=np.float32).reshape(-1)[0])
    ssum = np.zeros((NSEG, 1), dtype=np.float32)
    for core in range(NCORES):
        s, e = int(bounds[core]), int(bounds[core + 1])
        n = e - s
        p = np.exp(scores[core].reshape(-1)[:n].astype(np.float32) + b2v)
        seg = batch[s:e]
        ssum[:, 0] += np.bincount(seg, weights=p, minlength=NSEG).astype(np.float32)
    return ssum


def kernel(x, batch, W1, b1, W2, b2):
    batch = np.asarray(batch)
    chunks, in_maps = _prep_inputs(x, batch, W1, b1, W2, b2)
    try:
        res = _runner(chunks)(in_maps)
        wx = res["out"].reshape(NSEG, HID)
        scores = res["scores"]
    except Exception:
        # fall back to the stock SPMD driver (recompiles per call)
        from concourse.bass_utils import run_bass_kernel_spmd
        r = run_bass_kernel_spmd(_compiled(chunks), in_maps,
                                 core_ids=list(range(NCORES)))
        wx = np.concatenate([r.results[i]["out"] for i in range(NCORES)], axis=0)
        scores = np.stack([r.results[i]["scores"] for i in range(NCORES)])
    ssum = _host_ssum(scores, batch, b2)
    out = np.divide(wx, ssum, out=np.zeros_like(wx), where=ssum != 0)
    return out.astype(np.float32)


# revision 25
# speedup vs baseline: 1.0697x; 1.0697x over previous
"""AttentionPooling (segment softmax-weighted scatter) Trainium2 Bass kernel.

Strategy (8 NeuronCores, SPMD):
  - Shard by SEGMENT BLOCKS: core c owns segments [c*128, (c+1)*128) and all
    nodes whose (sorted) batch id falls in that range. No cross-core reduction
    is needed: each segment lives entirely on one core. Host pads each core's
    node count to a common T tiles of 128 so the compiled program is identical
    across cores.
  - Scores are computed without max-subtraction: p = exp(s + b2) directly.
    Scores are bounded (|tanh|<=1, |W2| small) so raw exp is safe in fp32,
    and the reference's +1e-8 epsilon is negligible relative to seg sums.
  - Phase A: s = tanh(x @ W1 + b1) @ W2 from a host-fed TRANSPOSED fp8 copy
    of 16*x. The 256-deep contraction runs as ONE DoubleRow fp8 matmul
    (hid-half pair on the DR axis) plus a second DR matmul against the fp8
    residual of 16*W1, restoring full W1 precision. tanh rescales by 1/256.
  - W2 scores for 4 consecutive chunks are steered to partitions 0..3 of one
    [4, F] PSUM tile (lhsT = w2 embedded in column j of a [H2, 4] zero block),
    one accumulation group, then a single DVE copy moves all 4 rows to SBUF.
  - Scores transpose to per-node columns directly from that SBUF tile (PE
    transposes, no DRAM round-trip); exp(s + b2) -> p_cols. scores also DMA
    to DRAM only for the host-side segment-sum export.
  - Phase B: one-hot weighted scatter. S[i, m] = (m == c_i) * p_i built by one
    dual-op tensor_scalar; out[seg, :] += S^T @ x accumulates in PSUM over all
    T tiles (natural-layout bf16 x, fed in X4G-tile batched DMAs with paced
    prefetch so the DMA engines never starve).
  - Pipelining: uniform SB-chunk blocks; block b's phase-A chunks interleave
    with block b-1's phase-B tiles (FB tiles per chunk), so scatter lags
    scores by only SB chunks at both ends of the program.
  - Host divides wx by the sum column and concatenates core outputs.
"""

from functools import lru_cache

import ml_dtypes
import numpy as np

import concourse.mybir as mybir
import concourse.tile as tile
from concourse import bacc
from concourse.masks import make_identity

P = 128          # partitions / tile rows
HID = 256        # hidden dim
H2 = 128         # MLP inner dim
NSEG = 1024      # segments (batch size)
NCORES = 8
F = 512          # phase-A chunk (nodes per score chunk)
FB = F // P      # tiles per chunk

BF16 = mybir.dt.bfloat16
FP8 = mybir.dt.float8e4
F32 = mybir.dt.float32
NPBF16 = ml_dtypes.bfloat16
NPFP8 = ml_dtypes.float8_e4m3
XSCALE = 16.0    # x and W1 are fed as fp8(16*v); tanh rescales by 1/256

SB = 16          # chunks per block / sub-bridge
XTP = 8          # chunks per xT DMA
X4G = 8          # node-tiles per natural-x DMA
X4_BUFS = 16     # natural-x tile buffers ([P, X4G*HID] bf16 each)
KB = 4           # x4-groups of phase-B emitted per phase-A block


def tile_order(chunks: int) -> np.ndarray:
    """Device iteration order: tile index t for each phase-B step j."""
    sb = min(SB, chunks)
    j = np.arange(chunks * FB)
    blk, s = j // (FB * sb), j % (FB * sb)
    fb, nl = s // sb, s % sb
    return FB * (blk * sb + nl) + fb


def build_kernel(chunks: int):
    assert chunks % SB == 0 and chunks % XTP == 0 and chunks <= P
    NB = chunks // SB
    T = chunks * FB
    n_pad = T * P

    nc = bacc.Bacc("TRN2")
    # natural x, X4G node-tiles per row: x[j, p, i*HID:(i+1)*HID] is lane p of
    # node-tile t_order[X4G*j + i]
    x_in = nc.dram_tensor("x", [T // X4G, P, X4G * HID], BF16,
                          kind="ExternalInput")
    # xT fp8: [128(k), chunks, 2(hid half i), F] = fp8(16*x[g*F+f, i*128+k])
    xT_in = nc.dram_tensor("xT", [P, chunks, 2, F], FP8, kind="ExternalInput")
    c_in = nc.dram_tensor("c", [P, T], F32, kind="ExternalInput")
    # w1: [2(hi/residual), 128(k), 2(hid half i), H2] fp8 of 16*W1 (+ residual)
    w1_in = nc.dram_tensor("w1", [2, P, 2, H2], FP8, kind="ExternalInput")
    # w2 on the diagonal of SB [H2, SB] blocks: lhsT w2_in[:, j] steers chunk
    # j%SB's scores to PSUM partition j of a shared [SB, F] tile.
    w2_in = nc.dram_tensor("w2", [H2, SB, SB], BF16, kind="ExternalInput")
    b1_in = nc.dram_tensor("b1", [H2, 1], F32, kind="ExternalInput")
    b2_in = nc.dram_tensor("b2", [P, 1], F32, kind="ExternalInput")
    iota_in = nc.dram_tensor("iota", [P, P], BF16, kind="ExternalInput")
    out_t = nc.dram_tensor("out", [P, HID], F32, kind="ExternalOutput")
    scores_d = nc.dram_tensor("scores", [chunks, F], F32, kind="ExternalOutput")

    with tile.TileContext(nc) as tc:
        with (
            tc.tile_pool(name="const", bufs=1) as cpool,
            tc.tile_pool(name="xT", bufs=6) as xT_pool,
            tc.tile_pool(name="th", bufs=4) as th_pool,
            tc.tile_pool(name="b16", bufs=3) as b_pool,
            tc.tile_pool(name="x4", bufs=X4_BUFS) as x4_pool,
            tc.tile_pool(name="S", bufs=32) as s_pool,
            tc.tile_pool(name="ph", bufs=3, space="PSUM") as ph_pool,
            tc.tile_pool(name="ps", bufs=2, space="PSUM") as ps_pool,
            tc.tile_pool(name="pT", bufs=1, space="PSUM") as pT_pool,
            tc.tile_pool(name="po", bufs=1, space="PSUM") as po_pool,
        ):
            # ---- constants ----
            w1h = cpool.tile([P, 2, H2], FP8, tag="w1h")
            w1r = cpool.tile([P, 2, H2], FP8, tag="w1r")
            w2q = cpool.tile([H2, SB, SB], BF16, tag="w2q")
            b1t = cpool.tile([H2, 1], F32, tag="b1t")
            b2t = cpool.tile([P, 1], F32, tag="b2t")
            iota_t = cpool.tile([P, P], BF16, tag="iota")
            ident = cpool.tile([P, P], F32, tag="ident")
            c_cols = cpool.tile([P, T], F32, tag="ccols")
            p_cols = cpool.tile([P, T], F32, tag="pcols")
            out_sb = cpool.tile([P, HID], F32, tag="osb")

            nc.gpsimd.dma_start(out=w1h[:], in_=w1_in[0])
            nc.gpsimd.dma_start(out=w1r[:], in_=w1_in[1])
            nc.gpsimd.dma_start(out=w2q[:], in_=w2_in[:])
            nc.gpsimd.dma_start(out=b1t[:], in_=b1_in[:])
            nc.gpsimd.dma_start(out=b2t[:], in_=b2_in[:])
            nc.gpsimd.dma_start(out=iota_t[:], in_=iota_in[:])
            nc.gpsimd.dma_start(out=c_cols[:], in_=c_in[:])
            make_identity(nc, ident[:])

            po = po_pool.tile([P, HID], F32)

            st = {"x4_load": 0, "x4": {}, "xT_load": 0, "xT": {},
                  "ps": None, "b16": None, "done": 0}

            def ensure_x4(upto: int, max_issue: int = 2):
                lim = min(upto, T // X4G, st["x4_load"] + max_issue)
                while st["x4_load"] < lim:
                    t = x4_pool.tile([P, X4G * HID], BF16, tag="x4", name="x4")
                    nc.sync.dma_start(out=t[:], in_=x_in[st["x4_load"]])
                    st["x4"][st["x4_load"]] = t
                    st["x4_load"] += 1

            def ensure_xT(upto: int, max_issue: int = 2):
                lim = min(upto, chunks // XTP, st["xT_load"] + max_issue)
                while st["xT_load"] < lim:
                    t = xT_pool.tile([P, XTP, 2, F], FP8, tag="xT", name="xT")
                    nc.sync.dma_start(
                        out=t[:],
                        in_=xT_in[:, st["xT_load"] * XTP:(st["xT_load"] + 1) * XTP])
                    st["xT"][st["xT_load"]] = t
                    st["xT_load"] += 1

            def phase_a_chunk(g):
                if g % XTP == 0:
                    ensure_xT(g // XTP + 2)
                    if g // XTP - 1 in st["xT"]:
                        del st["xT"][g // XTP - 1]
                xt = st["xT"][g // XTP]
                ph = ph_pool.tile([P, F], F32)
                nc.tensor.matmul(out=ph[:], lhsT=w1h[:], rhs=xt[:, g % XTP],
                                 start=True, stop=False,
                                 perf_mode=mybir.MatmulPerfMode.DoubleRow)
                nc.tensor.matmul(out=ph[:], lhsT=w1r[:], rhs=xt[:, g % XTP],
                                 start=False, stop=True,
                                 perf_mode=mybir.MatmulPerfMode.DoubleRow)
                del xt
                th = th_pool.tile([P, F], BF16)
                nc.scalar.activation(out=th[:], in_=ph[:],
                                     func=mybir.ActivationFunctionType.Tanh,
                                     bias=b1t[:], scale=1.0 / (XSCALE * XSCALE))
                j = g % SB
                if j == 0:
                    st["ps"] = ps_pool.tile([SB, F], F32, tag="ps", name="ps")
                ps = st["ps"]
                nc.tensor.matmul(out=ps[:], lhsT=w2q[:, j], rhs=th[:],
                                 start=(j == 0), stop=(j == SB - 1),
                                 skip_group_check=True)
                if j == SB - 1:
                    st["b16"] = b_pool.tile([SB, F], F32, tag="b16",
                                            name="b16")
                    nc.vector.tensor_copy(out=st["b16"][:], in_=ps[:])

            def sub_bridge(b):
                """Block b's scores -> p_cols (and DRAM export for the host)."""
                g_lo = b * SB
                b16 = st["b16"]
                nc.gpsimd.dma_start(out=scores_d[g_lo:g_lo + SB, :], in_=b16[:])
                pT = pT_pool.tile([P, FB * SB], F32)
                for fb in range(FB):
                    nc.tensor.transpose(
                        out=pT[:, fb * SB:(fb + 1) * SB],
                        in_=b16[:, fb * P:(fb + 1) * P],
                        identity=ident[:SB, :SB])
                nc.scalar.activation(
                    out=p_cols[:, g_lo * FB:(g_lo + SB) * FB], in_=pT[:],
                    func=mybir.ActivationFunctionType.Exp, bias=b2t[:],
                    scale=1.0)

            def phase_b_tiles(js):
                for j in js:
                    grp = j // X4G
                    x4 = st["x4"][grp]
                    if j % X4G == X4G - 1:
                        del st["x4"][grp]
                    i8 = j % X4G
                    S = s_pool.tile([P, P], BF16, tag="S")
                    nc.vector.tensor_scalar(
                        out=S[:], in0=iota_t[:],
                        scalar1=c_cols[:, j:j + 1], scalar2=p_cols[:, j:j + 1],
                        op0=mybir.AluOpType.is_equal, op1=mybir.AluOpType.mult)
                    nc.tensor.matmul(out=po[:], lhsT=S[:],
                                     rhs=x4[:, i8 * HID:(i8 + 1) * HID],
                                     start=(j == 0), stop=(j == T - 1),
                                     skip_group_check=True)

            def emit_b_group():
                d = st["done"]
                if d >= T:
                    return
                ensure_x4(d // X4G + 8)
                phase_b_tiles(range(d, d + X4G))
                st["done"] = d + X4G

            # Phase A front-loaded (Act/DMA paced) with just enough phase B
            # interleaved to keep the DMA engines fed; the rest of phase B
            # streams afterwards at x4-DMA pace.
            for b in range(NB):
                for gi in range(SB):
                    phase_a_chunk(b * SB + gi)
                    if gi == 1 and b > 0:
                        sub_bridge(b - 1)
                    if b > 0 and gi % 4 == 3:
                        emit_b_group()
            sub_bridge(NB - 1)
            while st["done"] < T:
                emit_b_group()

            nc.vector.tensor_copy(out=out_sb[:], in_=po[:])
            nc.gpsimd.dma_start(out=out_t[:], in_=out_sb[:])

    nc.finalize()
    return nc


@lru_cache(maxsize=4)
def _compiled(chunks: int):
    return build_kernel(chunks)


@lru_cache(maxsize=4)
def _runner(chunks: int):
    """Persistent jitted shard_map over the 8 cores (compiles once)."""
    import jax
    from concourse import bass2jax
    from jax.sharding import Mesh, PartitionSpec
    from jax.experimental.shard_map import shard_map

    nc = _compiled(chunks)
    bass2jax.install_neuronx_cc_hook()
    partition_name = nc.partition_id_tensor.name if nc.partition_id_tensor else None
    in_names, out_names, out_avals, zero_outs = [], [], [], []
    for alloc in nc.m.functions[0].allocations:
        if not isinstance(alloc, mybir.MemoryLocationSet):
            continue
        name = alloc.memorylocations[0].name
        if alloc.kind == "ExternalInput":
            if name != partition_name:
                in_names.append(name)
        elif alloc.kind == "ExternalOutput":
            out_names.append(name)
            shape = tuple(alloc.tensor_shape)
            dtype = mybir.dt.np(alloc.dtype)
            out_avals.append(jax.core.ShapedArray(shape, dtype))
            zero_outs.append(np.zeros(shape, dtype))
    n_params = len(in_names)
    all_in_names = list(in_names) + list(out_names)
    if partition_name is not None:
        all_in_names.append(partition_name)

    def _body(*args):
        operands = list(args)
        if partition_name is not None:
            operands.append(bass2jax.partition_id_tensor())
        outs = bass2jax._bass_exec_p.bind(
            *operands,
            out_avals=tuple(out_avals),
            in_names=tuple(all_in_names),
            out_names=tuple(out_names),
            lowering_input_output_aliases=(),
            sim_require_finite=True,
            sim_require_nnan=True,
            nc=nc,
        )
        return tuple(outs)

    devices = jax.devices()[:NCORES]
    assert len(devices) >= NCORES
    mesh = Mesh(np.asarray(devices), ("core",))
    in_specs = (PartitionSpec("core"),) * (n_params + len(out_names))
    out_specs = (PartitionSpec("core"),) * len(out_names)
    sharded = jax.jit(
        shard_map(_body, mesh=mesh, in_specs=in_specs, out_specs=out_specs,
                  check_rep=False),
        keep_unused=True,
    )
    concat_zeros = [
        np.zeros((NCORES * z.shape[0], *z.shape[1:]), z.dtype) for z in zero_outs
    ]

    def run(in_maps):
        concat_in = [
            np.concatenate([np.asarray(in_maps[c][n]) for c in range(NCORES)],
                           axis=0)
            for n in in_names
        ]
        out = sharded(*concat_in, *concat_zeros)
        return {
            name: np.asarray(out[i]).reshape(NCORES, *out_avals[i].shape)
            for i, name in enumerate(out_names)
        }

    return run


def _prep_inputs(x, batch, W1, b1, W2, b2):
    """Shard by segment blocks; build padded per-core arrays."""
    x = np.asarray(x, dtype=np.float32)
    batch = np.asarray(batch).astype(np.int64)
    bounds = np.searchsorted(batch, np.arange(0, NSEG + 1, P))
    counts = np.diff(bounds)
    maxn = int(counts.max())
    chunks = -(-maxn // F)
    chunks = -(-chunks // SB) * SB  # block alignment
    assert chunks <= P, f"core node count {maxn} exceeds capacity"
    T = chunks * FB
    n_pad = T * P
    t_order = tile_order(chunks)

    x_dev = np.zeros((NCORES, T // X4G, P, X4G * HID), dtype=NPBF16)
    xT_dev = np.zeros((NCORES, P, chunks, 2, F), dtype=NPFP8)
    c_dev = np.empty((NCORES, P, T), dtype=np.float32)
    for core in range(NCORES):
        s, e = int(bounds[core]), int(bounds[core + 1])
        n = e - s
        xs = x[s:e].astype(NPBF16)
        x_pad = np.zeros((n_pad, HID), dtype=NPBF16)
        x_pad[:n] = xs
        x_dev[core] = (x_pad.reshape(T, P, HID)[t_order]
                       .reshape(T // X4G, X4G, P, HID)
                       .transpose(0, 2, 1, 3)
                       .reshape(T // X4G, P, X4G * HID))
        x8_pad = np.zeros((n_pad, HID), dtype=NPFP8)
        x8_pad[:n] = (XSCALE * x[s:e]).astype(NPFP8)
        # [node(g,f), hid(i,k)] -> [k, g, i, f]
        xT_dev[core] = (x8_pad.reshape(chunks, F, 2, P)
                        .transpose(3, 0, 2, 1))
        c_all = np.full(n_pad, -1000.0, dtype=np.float32)
        c_all[:n] = (batch[s:e] - core * P).astype(np.float32)
        c_dev[core] = c_all.reshape(T, P)[t_order].T

    w1s = XSCALE * np.asarray(W1, dtype=np.float32)
    w1hi = w1s.astype(NPFP8)
    w1re = (w1s - w1hi.astype(np.float32)).astype(NPFP8)
    # [hid(i,k), m] -> [hi/res, k, i, m]
    w1 = (np.stack([w1hi, w1re])
          .reshape(2, 2, P, H2).transpose(0, 2, 1, 3))
    w2b = np.asarray(W2, dtype=np.float32).astype(NPBF16).reshape(H2)
    w2 = np.zeros((H2, SB, SB), dtype=NPBF16)
    for j in range(SB):
        w2[:, j, j] = w2b
    b1c = np.asarray(b1, dtype=np.float32).reshape(H2, 1)
    b2c = np.full((P, 1), np.float32(np.asarray(b2).reshape(-1)[0]))
    iota = np.broadcast_to(np.arange(P, dtype=np.float32), (P, P)).astype(NPBF16)

    in_maps = []
    for core in range(NCORES):
        in_maps.append({
            "x": x_dev[core], "xT": xT_dev[core], "c": c_dev[core],
            "w1": w1, "w2": w2, "b1": b1c, "b2": b2c, "iota": iota,
        })
    return chunks, in_maps


def _host_ssum(scores, batch, b2):
    """Per-segment sum of p = exp(score + b2), from exported per-core scores.

    scores[core] is [chunks, F] over that core's padded node stream; entry
    (g, f) is node g*F + f of the core's stream. Padded nodes are excluded by
    counting only the first n_c real nodes."""
    batch = np.asarray(batch).astype(np.int64)
    bounds = np.searchsorted(batch, np.arange(0, NSEG + 1, P))
    b2v = np.float32(np.asarray(b2, dtype=np.float32).reshape(-1)[0])
    ssum = np.zeros((NSEG, 1), dtype=np.float32)
    for core in range(NCORES):
        s, e = int(bounds[core]), int(bounds[core + 1])
        n = e - s
        p = np.exp(scores[core].reshape(-1)[:n].astype(np.float32) + b2v)
        seg = batch[s:e]
        ssum[:, 0] += np.bincount(seg, weights=p, minlength=NSEG).astype(np.float32)
    return ssum


def kernel(x, batch, W1, b1, W2, b2):
    batch = np.asarray(batch)
    chunks, in_maps = _prep_inputs(x, batch, W1, b1, W2, b2)
    try:
        res = _runner(chunks)(in_maps)
        wx = res["out"].reshape(NSEG, HID)
        scores = res["scores"]
    except Exception:
        # fall back to the stock SPMD driver (recompiles per call)
        from concourse.bass_utils import run_bass_kernel_spmd
        r = run_bass_kernel_spmd(_compiled(chunks), in_maps,
                                 core_ids=list(range(NCORES)))
        wx = np.concatenate([r.results[i]["out"] for i in range(NCORES)], axis=0)
        scores = np.stack([r.results[i]["scores"] for i in range(NCORES)])
    ssum = _host_ssum(scores, batch, b2)
    out = np.divide(wx, ssum, out=np.zeros_like(wx), where=ssum != 0)
    return out.astype(np.float32)


# revision 26
# speedup vs baseline: 1.0823x; 1.0118x over previous
"""AttentionPooling (segment softmax-weighted scatter) Trainium2 Bass kernel.

Strategy (8 NeuronCores, SPMD):
  - Shard by SEGMENT BLOCKS: core c owns segments [c*128, (c+1)*128) and all
    nodes whose (sorted) batch id falls in that range. No cross-core reduction
    is needed: each segment lives entirely on one core. Host pads each core's
    node count to a common T tiles of 128 so the compiled program is identical
    across cores.
  - Scores are computed without max-subtraction: p = exp(s + b2) directly.
    Scores are bounded (|tanh|<=1, |W2| small) so raw exp is safe in fp32,
    and the reference's +1e-8 epsilon is negligible relative to seg sums.
  - Phase A: s = tanh(x @ W1 + b1) @ W2 from a host-fed TRANSPOSED fp8 copy
    of 16*x. The 256-deep contraction runs as ONE DoubleRow fp8 matmul
    (hid-half pair on the DR axis) plus a second DR matmul against the fp8
    residual of 16*W1, restoring full W1 precision. tanh rescales by 1/256.
  - W2 scores for 4 consecutive chunks are steered to partitions 0..3 of one
    [4, F] PSUM tile (lhsT = w2 embedded in column j of a [H2, 4] zero block),
    one accumulation group, then a single DVE copy moves all 4 rows to SBUF.
  - Scores transpose to per-node columns directly from that SBUF tile (PE
    transposes, no DRAM round-trip); exp(s + b2) -> p_cols. scores also DMA
    to DRAM only for the host-side segment-sum export.
  - Phase B: one-hot weighted scatter. S[i, m] = (m == c_i) * p_i built by one
    dual-op tensor_scalar; out[seg, :] += S^T @ x accumulates in PSUM over all
    T tiles (natural-layout bf16 x, fed in X4G-tile batched DMAs with paced
    prefetch so the DMA engines never starve).
  - Pipelining: uniform SB-chunk blocks; block b's phase-A chunks interleave
    with block b-1's phase-B tiles (FB tiles per chunk), so scatter lags
    scores by only SB chunks at both ends of the program.
  - Host divides wx by the sum column and concatenates core outputs.
"""

from functools import lru_cache

import ml_dtypes
import numpy as np

import concourse.mybir as mybir
import concourse.tile as tile
from concourse import bacc
from concourse.masks import make_identity

P = 128          # partitions / tile rows
HID = 256        # hidden dim
H2 = 128         # MLP inner dim
NSEG = 1024      # segments (batch size)
NCORES = 8
F = 512          # phase-A chunk (nodes per score chunk)
FB = F // P      # tiles per chunk

BF16 = mybir.dt.bfloat16
FP8 = mybir.dt.float8e4
F32 = mybir.dt.float32
NPBF16 = ml_dtypes.bfloat16
NPFP8 = ml_dtypes.float8_e4m3
XSCALE = 16.0    # x and W1 are fed as fp8(16*v); tanh rescales by 1/256

SB = 16          # chunks per block / sub-bridge
XTP = 8          # chunks per xT DMA
X4G = 8          # node-tiles per natural-x DMA
X4_BUFS = 16     # natural-x tile buffers ([P, X4G*HID] bf16 each)
KB = 4           # x4-groups of phase-B emitted per phase-A block


def tile_order(chunks: int) -> np.ndarray:
    """Device iteration order: tile index t for each phase-B step j."""
    sb = min(SB, chunks)
    j = np.arange(chunks * FB)
    blk, s = j // (FB * sb), j % (FB * sb)
    fb, nl = s // sb, s % sb
    return FB * (blk * sb + nl) + fb


def build_kernel(chunks: int):
    assert chunks % SB == 0 and chunks % XTP == 0 and chunks <= P
    NB = chunks // SB
    T = chunks * FB
    n_pad = T * P

    nc = bacc.Bacc("TRN2")
    # natural x, X4G node-tiles per row: x[j, p, i*HID:(i+1)*HID] is lane p of
    # node-tile t_order[X4G*j + i]
    x_in = nc.dram_tensor("x", [T // X4G, P, X4G * HID], BF16,
                          kind="ExternalInput")
    # xT fp8: [128(k), chunks, 2(hid half i), F] = fp8(16*x[g*F+f, i*128+k])
    xT_in = nc.dram_tensor("xT", [P, chunks, 2, F], FP8, kind="ExternalInput")
    c_in = nc.dram_tensor("c", [P, T], F32, kind="ExternalInput")
    # w1: [2(hi/residual), 128(k), 2(hid half i), H2] fp8 of 16*W1 (+ residual)
    w1_in = nc.dram_tensor("w1", [2, P, 2, H2], FP8, kind="ExternalInput")
    # w2 on the diagonal of SB [H2, SB] blocks: lhsT w2_in[:, j] steers chunk
    # j%SB's scores to PSUM partition j of a shared [SB, F] tile.
    w2_in = nc.dram_tensor("w2", [H2, SB, SB], BF16, kind="ExternalInput")
    b1_in = nc.dram_tensor("b1", [H2, 1], F32, kind="ExternalInput")
    b2_in = nc.dram_tensor("b2", [P, 1], F32, kind="ExternalInput")
    iota_in = nc.dram_tensor("iota", [P, P], BF16, kind="ExternalInput")
    out_t = nc.dram_tensor("out", [P, HID], F32, kind="ExternalOutput")
    scores_d = nc.dram_tensor("scores", [chunks, F], F32, kind="ExternalOutput")

    with tile.TileContext(nc) as tc:
        with (
            tc.tile_pool(name="const", bufs=1) as cpool,
            tc.tile_pool(name="xT", bufs=6) as xT_pool,
            tc.tile_pool(name="th", bufs=4) as th_pool,
            tc.tile_pool(name="b16", bufs=3) as b_pool,
            tc.tile_pool(name="x4", bufs=X4_BUFS) as x4_pool,
            tc.tile_pool(name="S", bufs=64) as s_pool,
            tc.tile_pool(name="ph", bufs=3, space="PSUM") as ph_pool,
            tc.tile_pool(name="ps", bufs=2, space="PSUM") as ps_pool,
            tc.tile_pool(name="pT", bufs=1, space="PSUM") as pT_pool,
            tc.tile_pool(name="po", bufs=1, space="PSUM") as po_pool,
        ):
            # ---- constants ----
            w1h = cpool.tile([P, 2, H2], FP8, tag="w1h")
            w1r = cpool.tile([P, 2, H2], FP8, tag="w1r")
            w2q = cpool.tile([H2, SB, SB], BF16, tag="w2q")
            b1t = cpool.tile([H2, 1], F32, tag="b1t")
            b2t = cpool.tile([P, 1], F32, tag="b2t")
            iota_t = cpool.tile([P, P], BF16, tag="iota")
            ident = cpool.tile([P, P], F32, tag="ident")
            c_cols = cpool.tile([P, T], F32, tag="ccols")
            p_cols = cpool.tile([P, T], F32, tag="pcols")
            out_sb = cpool.tile([P, HID], F32, tag="osb")

            nc.gpsimd.dma_start(out=w1h[:], in_=w1_in[0])
            nc.gpsimd.dma_start(out=w1r[:], in_=w1_in[1])
            nc.gpsimd.dma_start(out=w2q[:], in_=w2_in[:])
            nc.gpsimd.dma_start(out=b1t[:], in_=b1_in[:])
            nc.gpsimd.dma_start(out=b2t[:], in_=b2_in[:])
            nc.gpsimd.dma_start(out=iota_t[:], in_=iota_in[:])
            nc.gpsimd.dma_start(out=c_cols[:], in_=c_in[:])
            make_identity(nc, ident[:])

            po = po_pool.tile([P, HID], F32)

            st = {"x4_load": 0, "x4": {}, "xT_load": 0, "xT": {},
                  "ps": None, "b16": None, "done": 0}

            def ensure_x4(upto: int, max_issue: int = 2):
                lim = min(upto, T // X4G, st["x4_load"] + max_issue)
                while st["x4_load"] < lim:
                    t = x4_pool.tile([P, X4G * HID], BF16, tag="x4", name="x4")
                    nc.sync.dma_start(out=t[:], in_=x_in[st["x4_load"]])
                    st["x4"][st["x4_load"]] = t
                    st["x4_load"] += 1

            def ensure_xT(upto: int, max_issue: int = 2):
                lim = min(upto, chunks // XTP, st["xT_load"] + max_issue)
                while st["xT_load"] < lim:
                    t = xT_pool.tile([P, XTP, 2, F], FP8, tag="xT", name="xT")
                    nc.sync.dma_start(
                        out=t[:],
                        in_=xT_in[:, st["xT_load"] * XTP:(st["xT_load"] + 1) * XTP])
                    st["xT"][st["xT_load"]] = t
                    st["xT_load"] += 1

            def phase_a_chunk(g):
                if g % XTP == 0:
                    ensure_xT(g // XTP + 2)
                    if g // XTP - 1 in st["xT"]:
                        del st["xT"][g // XTP - 1]
                xt = st["xT"][g // XTP]
                ph = ph_pool.tile([P, F], F32)
                nc.tensor.matmul(out=ph[:], lhsT=w1h[:], rhs=xt[:, g % XTP],
                                 start=True, stop=False,
                                 perf_mode=mybir.MatmulPerfMode.DoubleRow)
                nc.tensor.matmul(out=ph[:], lhsT=w1r[:], rhs=xt[:, g % XTP],
                                 start=False, stop=True,
                                 perf_mode=mybir.MatmulPerfMode.DoubleRow)
                del xt
                th = th_pool.tile([P, F], BF16)
                nc.scalar.activation(out=th[:], in_=ph[:],
                                     func=mybir.ActivationFunctionType.Tanh,
                                     bias=b1t[:], scale=1.0 / (XSCALE * XSCALE))
                j = g % SB
                if j == 0:
                    st["ps"] = ps_pool.tile([SB, F], F32, tag="ps", name="ps")
                ps = st["ps"]
                nc.tensor.matmul(out=ps[:], lhsT=w2q[:, j], rhs=th[:],
                                 start=(j == 0), stop=(j == SB - 1),
                                 skip_group_check=True)
                if j == SB - 1:
                    st["b16"] = b_pool.tile([SB, F], F32, tag="b16",
                                            name="b16")
                    nc.vector.tensor_copy(out=st["b16"][:], in_=ps[:])

            def sub_bridge(b):
                """Block b's scores -> p_cols (and DRAM export for the host)."""
                g_lo = b * SB
                b16 = st["b16"]
                nc.gpsimd.dma_start(out=scores_d[g_lo:g_lo + SB, :], in_=b16[:])
                pT = pT_pool.tile([P, FB * SB], F32)
                for fb in range(FB):
                    nc.tensor.transpose(
                        out=pT[:, fb * SB:(fb + 1) * SB],
                        in_=b16[:, fb * P:(fb + 1) * P],
                        identity=ident[:SB, :SB])
                nc.scalar.activation(
                    out=p_cols[:, g_lo * FB:(g_lo + SB) * FB], in_=pT[:],
                    func=mybir.ActivationFunctionType.Exp, bias=b2t[:],
                    scale=1.0)

            def phase_b_tiles(js):
                for j in js:
                    grp = j // X4G
                    x4 = st["x4"][grp]
                    if j % X4G == X4G - 1:
                        del st["x4"][grp]
                    i8 = j % X4G
                    S = s_pool.tile([P, P], BF16, tag="S")
                    nc.vector.tensor_scalar(
                        out=S[:], in0=iota_t[:],
                        scalar1=c_cols[:, j:j + 1], scalar2=p_cols[:, j:j + 1],
                        op0=mybir.AluOpType.is_equal, op1=mybir.AluOpType.mult)
                    nc.tensor.matmul(out=po[:], lhsT=S[:],
                                     rhs=x4[:, i8 * HID:(i8 + 1) * HID],
                                     start=(j == 0), stop=(j == T - 1),
                                     skip_group_check=True)

            def emit_b_group():
                d = st["done"]
                if d >= T:
                    return
                ensure_x4(d // X4G + 8)
                phase_b_tiles(range(d, d + X4G))
                st["done"] = d + X4G

            # Phase A front-loaded (Act/DMA paced) with just enough phase B
            # interleaved to keep the DMA engines fed; the rest of phase B
            # streams afterwards at x4-DMA pace.
            for b in range(NB):
                for gi in range(SB):
                    phase_a_chunk(b * SB + gi)
                    if gi == 1 and b > 0:
                        sub_bridge(b - 1)
                    if b > 0 and gi % 4 == 3:
                        emit_b_group()
            sub_bridge(NB - 1)
            while st["done"] < T:
                emit_b_group()

            nc.vector.tensor_copy(out=out_sb[:], in_=po[:])
            nc.gpsimd.dma_start(out=out_t[:], in_=out_sb[:])

    nc.finalize()
    return nc


@lru_cache(maxsize=4)
def _compiled(chunks: int):
    return build_kernel(chunks)


@lru_cache(maxsize=4)
def _runner(chunks: int):
    """Persistent jitted shard_map over the 8 cores (compiles once)."""
    import jax
    from concourse import bass2jax
    from jax.sharding import Mesh, PartitionSpec
    from jax.experimental.shard_map import shard_map

    nc = _compiled(chunks)
    bass2jax.install_neuronx_cc_hook()
    partition_name = nc.partition_id_tensor.name if nc.partition_id_tensor else None
    in_names, out_names, out_avals, zero_outs = [], [], [], []
    for alloc in nc.m.functions[0].allocations:
        if not isinstance(alloc, mybir.MemoryLocationSet):
            continue
        name = alloc.memorylocations[0].name
        if alloc.kind == "ExternalInput":
            if name != partition_name:
                in_names.append(name)
        elif alloc.kind == "ExternalOutput":
            out_names.append(name)
            shape = tuple(alloc.tensor_shape)
            dtype = mybir.dt.np(alloc.dtype)
            out_avals.append(jax.core.ShapedArray(shape, dtype))
            zero_outs.append(np.zeros(shape, dtype))
    n_params = len(in_names)
    all_in_names = list(in_names) + list(out_names)
    if partition_name is not None:
        all_in_names.append(partition_name)

    def _body(*args):
        operands = list(args)
        if partition_name is not None:
            operands.append(bass2jax.partition_id_tensor())
        outs = bass2jax._bass_exec_p.bind(
            *operands,
            out_avals=tuple(out_avals),
            in_names=tuple(all_in_names),
            out_names=tuple(out_names),
            lowering_input_output_aliases=(),
            sim_require_finite=True,
            sim_require_nnan=True,
            nc=nc,
        )
        return tuple(outs)

    devices = jax.devices()[:NCORES]
    assert len(devices) >= NCORES
    mesh = Mesh(np.asarray(devices), ("core",))
    in_specs = (PartitionSpec("core"),) * (n_params + len(out_names))
    out_specs = (PartitionSpec("core"),) * len(out_names)
    sharded = jax.jit(
        shard_map(_body, mesh=mesh, in_specs=in_specs, out_specs=out_specs,
                  check_rep=False),
        keep_unused=True,
    )
    concat_zeros = [
        np.zeros((NCORES * z.shape[0], *z.shape[1:]), z.dtype) for z in zero_outs
    ]

    def run(in_maps):
        concat_in = [
            np.concatenate([np.asarray(in_maps[c][n]) for c in range(NCORES)],
                           axis=0)
            for n in in_names
        ]
        out = sharded(*concat_in, *concat_zeros)
        return {
            name: np.asarray(out[i]).reshape(NCORES, *out_avals[i].shape)
            for i, name in enumerate(out_names)
        }

    return run


def _prep_inputs(x, batch, W1, b1, W2, b2):
    """Shard by segment blocks; build padded per-core arrays."""
    x = np.asarray(x, dtype=np.float32)
    batch = np.asarray(batch).astype(np.int64)
    bounds = np.searchsorted(batch, np.arange(0, NSEG + 1, P))
    counts = np.diff(bounds)
    maxn = int(counts.max())
    chunks = -(-maxn // F)
    chunks = -(-chunks // SB) * SB  # block alignment
    assert chunks <= P, f"core node count {maxn} exceeds capacity"
    T = chunks * FB
    n_pad = T * P
    t_order = tile_order(chunks)

    x_dev = np.zeros((NCORES, T // X4G, P, X4G * HID), dtype=NPBF16)
    xT_dev = np.zeros((NCORES, P, chunks, 2, F), dtype=NPFP8)
    c_dev = np.empty((NCORES, P, T), dtype=np.float32)
    for core in range(NCORES):
        s, e = int(bounds[core]), int(bounds[core + 1])
        n = e - s
        xs = x[s:e].astype(NPBF16)
        x_pad = np.zeros((n_pad, HID), dtype=NPBF16)
        x_pad[:n] = xs
        x_dev[core] = (x_pad.reshape(T, P, HID)[t_order]
                       .reshape(T // X4G, X4G, P, HID)
                       .transpose(0, 2, 1, 3)
                       .reshape(T // X4G, P, X4G * HID))
        x8_pad = np.zeros((n_pad, HID), dtype=NPFP8)
        x8_pad[:n] = (XSCALE * x[s:e]).astype(NPFP8)
        # [node(g,f), hid(i,k)] -> [k, g, i, f]
        xT_dev[core] = (x8_pad.reshape(chunks, F, 2, P)
                        .transpose(3, 0, 2, 1))
        c_all = np.full(n_pad, -1000.0, dtype=np.float32)
        c_all[:n] = (batch[s:e] - core * P).astype(np.float32)
        c_dev[core] = c_all.reshape(T, P)[t_order].T

    w1s = XSCALE * np.asarray(W1, dtype=np.float32)
    w1hi = w1s.astype(NPFP8)
    w1re = (w1s - w1hi.astype(np.float32)).astype(NPFP8)
    # [hid(i,k), m] -> [hi/res, k, i, m]
    w1 = (np.stack([w1hi, w1re])
          .reshape(2, 2, P, H2).transpose(0, 2, 1, 3))
    w2b = np.asarray(W2, dtype=np.float32).astype(NPBF16).reshape(H2)
    w2 = np.zeros((H2, SB, SB), dtype=NPBF16)
    for j in range(SB):
        w2[:, j, j] = w2b
    b1c = np.asarray(b1, dtype=np.float32).reshape(H2, 1)
    b2c = np.full((P, 1), np.float32(np.asarray(b2).reshape(-1)[0]))
    iota = np.broadcast_to(np.arange(P, dtype=np.float32), (P, P)).astype(NPBF16)

    in_maps = []
    for core in range(NCORES):
        in_maps.append({
            "x": x_dev[core], "xT": xT_dev[core], "c": c_dev[core],
            "w1": w1, "w2": w2, "b1": b1c, "b2": b2c, "iota": iota,
        })
    return chunks, in_maps


def _host_ssum(scores, batch, b2):
    """Per-segment sum of p = exp(score + b2), from exported per-core scores.

    scores[core] is [chunks, F] over that core's padded node stream; entry
    (g, f) is node g*F + f of the core's stream. Padded nodes are excluded by
    counting only the first n_c real nodes."""
    batch = np.asarray(batch).astype(np.int64)
    bounds = np.searchsorted(batch, np.arange(0, NSEG + 1, P))
    b2v = np.float32(np.asarray(b2, dtype=np.float32).reshape(-1)[0])
    ssum = np.zeros((NSEG, 1), dtype=np.float32)
    for core in range(NCORES):
        s, e = int(bounds[core]), int(bounds[core + 1])
        n = e - s
        p = np.exp(scores[core].reshape(-1)[:n].astype(np.float32) + b2v)
        seg = batch[s:e]
        ssum[:, 0] += np.bincount(seg, weights=p, minlength=NSEG).astype(np.float32)
    return ssum


def kernel(x, batch, W1, b1, W2, b2):
    batch = np.asarray(batch)
    chunks, in_maps = _prep_inputs(x, batch, W1, b1, W2, b2)
    try:
        res = _runner(chunks)(in_maps)
        wx = res["out"].reshape(NSEG, HID)
        scores = res["scores"]
    except Exception:
        # fall back to the stock SPMD driver (recompiles per call)
        from concourse.bass_utils import run_bass_kernel_spmd
        r = run_bass_kernel_spmd(_compiled(chunks), in_maps,
                                 core_ids=list(range(NCORES)))
        wx = np.concatenate([r.results[i]["out"] for i in range(NCORES)], axis=0)
        scores = np.stack([r.results[i]["scores"] for i in range(NCORES)])
    ssum = _host_ssum(scores, batch, b2)
    out = np.divide(wx, ssum, out=np.zeros_like(wx), where=ssum != 0)
    return out.astype(np.float32)
